# revision 2
# baseline (speedup 1.0000x reference)
"""Single-head attention (B=8, D=1024, N=2048, fp32 I/O) on 8 TRN2 NeuronCores.

Sharding: data-parallel over batch — core i computes batch element i with the
full weights replicated. No collectives needed.

Per-core math (x: [D, N] features-first, W*: [D, D]):
    scores = x^T (W_q^T W_k) x / sqrt(D)        (Gram-matrix trick: no weight
    attn   = softmax(scores, axis=-1)            transposes needed on the Q/K
    out    = (W_v x) attn                        path)
  GT = W_k^T W_q              -> matmul(lhsT=W_k, rhs=W_q)     [f, e]
  U  = GT^T x = (W_q^T W_k) x -> matmul(lhsT=GT, rhs=x)        [e, m]
  S  = x^T U                  -> matmul(lhsT=x,  rhs=U)        [n, m]
  VT = x^T W_v^T              -> matmul(lhsT=x,  rhs=W_v^T)    [n, d]
  out= VT^T attn              -> matmul(lhsT=VT, rhs=attn)     [d, m]
The softmax normalizer 1/Z[n] is folded into VT's rows (n is the contraction
index of the output matmul), so attn is stored as unnormalized exp() in bf16.
Compute dtype bf16 (f32 PSUM accumulation); measured rel_l2 vs f32 ref ~5e-3.
"""

import numpy as np

import concourse.bacc as bacc
import concourse.mybir as mybir
import concourse.tile as tile
from concourse.bass_utils import run_bass_kernel_spmd
from concourse.masks import make_identity

B, D, N = 8, 1024, 2048
P = 128
CE = D // P   # 8 chunks on the feature axis
CN = N // P   # 16 chunks on the sequence axis
K_SCALE = 1.0 / float(np.sqrt(D))

F32 = mybir.dt.float32
BF16 = mybir.dt.bfloat16


def build_nc():
    nc = bacc.Bacc("TRN2", target_bir_lowering=False, debug=False)

    x_ext = nc.dram_tensor("x", [D, N], F32, kind="ExternalInput")
    wq_ext = nc.dram_tensor("W_q", [D, D], F32, kind="ExternalInput")
    wk_ext = nc.dram_tensor("W_k", [D, D], F32, kind="ExternalInput")
    wv_ext = nc.dram_tensor("W_v", [D, D], F32, kind="ExternalInput")
    out_ext = nc.dram_tensor("out", [D, N], F32, kind="ExternalOutput")

    x_re = x_ext.ap().rearrange("(c p) n -> c p n", p=P)
    wq_re = wq_ext.ap().rearrange("(c p) e -> c p e", p=P)
    wk_re = wk_ext.ap().rearrange("(c p) e -> c p e", p=P)
    wv_re = wv_ext.ap().rearrange("(c p) e -> c p e", p=P)
    out_re = out_ext.ap().rearrange("(c p) m -> c p m", p=P)

    with tile.TileContext(nc) as tc:
        with (
            tc.tile_pool(name="const", bufs=1) as const,
            tc.tile_pool(name="stage", bufs=3) as stage,
            tc.tile_pool(name="big", bufs=22) as big,
            tc.tile_pool(name="small", bufs=4) as small,
            tc.tile_pool(name="psum", bufs=3, space="PSUM") as psum,
        ):
            identity = const.tile([P, P], BF16)
            make_identity(nc, identity[:])
            # 1/Z per sequence position, [n_p, cn]
            recip_z = const.tile([P, CN], F32, tag="rz")

            # All big tensors share one 8KB/partition slot tag so SBUF slots
            # recycle across phases (peak ~20 live of 22 slots).
            wq_t =[big.tile([P, 4, 1024], BF16, tag="big", name=f"wq{i}") for i in range(2)]
            wk_t = [big.tile([P, 4, 1024], BF16, tag="big", name=f"wk{i}") for i in range(2)]
            wvt_t = [big.tile([P, 4, 1024], BF16, tag="big", name=f"wvt{i}") for i in range(2)]
            gt_t = [big.tile([P, 4, 1024], BF16, tag="big", name=f"gt{i}") for i in range(2)]
            x_t = [big.tile([P, 2, N], BF16, tag="big", name=f"x{i}") for i in range(4)]
            u_t = [big.tile([P, 2, N], BF16, tag="big", name=f"u{i}") for i in range(4)]
            vt_t = [big.tile([P, 4, 1024], BF16, tag="big", name=f"vt{i}") for i in range(4)]
            attn_t = [big.tile([P, 2, N], BF16, tag="big", name=f"at{i}") for i in range(8)]

            # ---- load + cast W_q, W_k --------------------------------------
            for w_re_, w_t in ((wq_re, wq_t), (wk_re, wk_t)):
                for c in range(CE):
                    st = stage.tile([P, D], F32, tag="stage")
                    nc.sync.dma_start(st[:], w_re_[c])
                    nc.vector.tensor_copy(w_t[c // 4][:, c % 4, :], st[:])

            # ---- GT = W_k^T W_q  [f, e] ------------------------------------
            for cf in range(CE):
                for et in range(2):
                    ps = psum.tile([P, 512], F32, tag="ps_s", bufs=2)
                    for dc in range(CE):
                        nc.tensor.matmul(
                            ps[:],
                            wk_t[dc // 4][:, dc % 4, cf * P:(cf + 1) * P],
                            wq_t[dc // 4][:, dc % 4, et * 512:(et + 1) * 512],
                            start=(dc == 0),
                            stop=(dc == CE - 1),
                        )
                    nc.vector.tensor_copy(gt_t[cf // 4][:, cf % 4, et * 512:(et + 1) * 512], ps[:])

            # ---- load + cast + transpose W_v -> WvT [e, d] -----------------
            for cd in range(CE):
                st = stage.tile([P, D], F32, tag="stage")
                nc.sync.dma_start(st[:], wv_re[cd])
                wvc = small.tile([P, D], BF16, tag="wvc", bufs=2)
                nc.vector.tensor_copy(wvc[:], st[:])
                for ce in range(CE):
                    pst = psum.tile([P, P], BF16, tag="ps_s", bufs=2)
                    nc.tensor.transpose(pst[:], wvc[:, ce * P:(ce + 1) * P], identity[:])
                    nc.vector.tensor_copy(
                        wvt_t[ce // 4][:, ce % 4, cd * P:(cd + 1) * P], pst[:]
                    )

            # ---- load + cast x [e, n] --------------------------------------
            for c in range(CE):
                st = stage.tile([P, N], F32, tag="stage")
                nc.sync.dma_start(st[:], x_re[c])
                nc.vector.tensor_copy(x_t[c // 2][:, c % 2, :], st[:])

            # ---- VT = x^T WvT  [n, d] --------------------------------------
            for cn in range(CN):
                for dt in range(2):
                    ps = psum.tile([P, 512], F32, tag="ps_s", bufs=2)
                    for ce in range(CE):
                        nc.tensor.matmul(
                            ps[:],
                            x_t[ce // 2][:, ce % 2, cn * P:(cn + 1) * P],
                            wvt_t[ce // 4][:, ce % 4, dt * 512:(dt + 1) * 512],
                            start=(ce == 0),
                            stop=(ce == CE - 1),
                        )
                    nc.vector.tensor_copy(vt_t[cn // 4][:, cn % 4, dt * 512:(dt + 1) * 512], ps[:])

            # ---- U = GT^T x = (W_q^T W_k) x  [e, m] ------------------------
            for ce in range(CE):
                for mt in range(4):
                    ps = psum.tile([P, 512], F32, tag="ps_s", bufs=2)
                    for cf in range(CE):
                        nc.tensor.matmul(
                            ps[:],
                            gt_t[cf // 4][:, cf % 4, ce * P:(ce + 1) * P],
                            x_t[cf // 2][:, cf % 2, mt * 512:(mt + 1) * 512],
                            start=(cf == 0),
                            stop=(cf == CE - 1),
                        )
                    nc.vector.tensor_copy(u_t[ce // 2][:, ce % 2, mt * 512:(mt + 1) * 512], ps[:])

            # ---- scores + softmax per 128-row chunk ------------------------
            for cn in range(CN):
                halves = []
                for h in range(2):
                    ph = psum.tile([P, 1024], F32, tag="ps_h", bufs=3)
                    for ms in range(2):
                        m0 = h * 1024 + ms * 512
                        for ce in range(CE):
                            nc.tensor.matmul(
                                ph[:, ms * 512:(ms + 1) * 512],
                                x_t[ce // 2][:, ce % 2, cn * P:(cn + 1) * P],
                                u_t[ce // 2][:, ce % 2, m0:m0 + 512],
                                start=(ce == 0),
                                stop=(ce == CE - 1),
                            )
                    halves.append(ph)
                mx0 = small.tile([P, 1], F32, tag="sm", bufs=4)
                mx1 = small.tile([P, 1], F32, tag="sm", bufs=4)
                nc.vector.reduce_max(mx0[:], halves[0][:], axis=mybir.AxisListType.X)
                nc.vector.reduce_max(mx1[:], halves[1][:], axis=mybir.AxisListType.X)
                bias = small.tile([P, 1], F32, tag="sm", bufs=4)
                nc.vector.tensor_max(bias[:], mx0[:], mx1[:])
                nc.vector.tensor_scalar_mul(bias[:], bias[:], -K_SCALE)
                z0 = small.tile([P, 1], F32, tag="sm", bufs=4)
                z1 = small.tile([P, 1], F32, tag="sm", bufs=4)
                for h, zh in ((0, z0), (1, z1)):
                    nc.scalar.activation(
                        attn_t[cn // 2][:, cn % 2, h * 1024:(h + 1) * 1024],
                        halves[h][:],
                        mybir.ActivationFunctionType.Exp,
                        bias=bias[:],
                        scale=K_SCALE,
                        accum_out=zh[:],
                    )
                nc.vector.tensor_add(z0[:], z0[:], z1[:])
                nc.vector.reciprocal(recip_z[:, cn:cn + 1], z0[:])

            # ---- fold 1/Z into VT rows -------------------------------------
            for cn in range(CN):
                nc.vector.tensor_scalar_mul(
                    vt_t[cn // 4][:, cn % 4, :],
                    vt_t[cn // 4][:, cn % 4, :],
                    recip_z[:, cn:cn + 1],
                )

            # ---- out = VTs^T attn  [d, m] ----------------------------------
            for dt in range(CE):
                ot = stage.tile([P, N], F32, tag="stage")
                for mt in range(4):
                    ps = psum.tile([P, 512], F32, tag="ps_s", bufs=2)
                    for cn in range(CN):
                        nc.tensor.matmul(
                            ps[:],
                            vt_t[cn // 4][:, cn % 4, dt * P:(dt + 1) * P],
                            attn_t[cn // 2][:, cn % 2, mt * 512:(mt + 1) * 512],
                            start=(cn == 0),
                            stop=(cn == CN - 1),
                        )
                    nc.vector.tensor_copy(ot[:, mt * 512:(mt + 1) * 512], ps[:])
                nc.sync.dma_start(out_re[dt], ot[:])

    nc.compile()
    return nc


_NC = None


def _get_nc():
    global _NC
    if _NC is None:
        _NC = build_nc()
    return _NC


def kernel(x, W_q, W_k, W_v):
    x = np.ascontiguousarray(np.asarray(x, dtype=np.float32))
    W_q = np.ascontiguousarray(np.asarray(W_q, dtype=np.float32))
    W_k = np.ascontiguousarray(np.asarray(W_k, dtype=np.float32))
    W_v = np.ascontiguousarray(np.asarray(W_v, dtype=np.float32))
    assert x.shape == (B, D, N), x.shape

    nc = _get_nc()
    in_maps = [
        {"x": x[i], "W_q": W_q, "W_k": W_k, "W_v": W_v} for i in range(B)
    ]
    res = run_bass_kernel_spmd(nc, in_maps, core_ids=list(range(B)))
    return np.stack([res.results[i]["out"] for i in range(B)], axis=0)


if __name__ == "__main__":
    rng = np.random.default_rng(0)
    scale = 1.0 / np.sqrt(D)
    x = rng.standard_normal((B, D, N), dtype=np.float32)
    wq = rng.standard_normal((D, D), dtype=np.float32) * scale
    wk = rng.standard_normal((D, D), dtype=np.float32) * scale
    wv = rng.standard_normal((D, D), dtype=np.float32) * scale
    out = kernel(x, wq, wk, wv)
    print("out", out.shape, out.dtype, np.abs(out).max())


# revision 4
# speedup vs baseline: 1.0188x; 1.0188x over previous
"""Single-head attention (B=8, D=1024, N=2048, fp32 I/O) on 8 TRN2 NeuronCores.

Sharding: data-parallel over batch — core i computes batch element i with the
full weights replicated. No collectives needed.

Per-core math (x: [D, N] features-first, W*: [D, D]):
    scores = x^T (W_q^T W_k) x / sqrt(D)        (Gram-matrix trick: no weight
    attn   = softmax(scores, axis=-1)            transposes needed on the Q/K
    out    = (W_v x) attn                        path)
  GT = W_k^T W_q              -> matmul(lhsT=W_k, rhs=W_q)     [f, e]
  U  = GT^T x = (W_q^T W_k) x -> matmul(lhsT=GT, rhs=x)        [e, m]
  S  = x^T U                  -> matmul(lhsT=x,  rhs=U)        [n, m]
  VT = x^T W_v^T              -> matmul(lhsT=x,  rhs=W_v^T)    [n, d]
  out= VT^T attn              -> matmul(lhsT=VT, rhs=attn)     [d, m]
W_v^T comes from an XBAR DMA transpose of the bf16 W_v via a DRAM scratch
(keeps the transpose off the TensorEngine). The softmax normalizer 1/Z[n] is
folded into VT's rows (n is the contraction index of the output matmul), so
attn is stored as unnormalized exp() in bf16. Compute dtype bf16 (f32 PSUM
accumulation); measured rel_l2 vs the f32 reference ~5e-3.

All PSUM tiles share one [128, 1024] (2-bank) tag, 4 bufs = all 8 banks; each
tile carries two interleaved 512-wide accumulation groups (separate banks) so
DMA-gated phases keep several matmuls issuable per arriving input chunk.
"""

import numpy as np

import concourse.bacc as bacc
import concourse.mybir as mybir
import concourse.tile as tile
from concourse.bass_utils import run_bass_kernel_spmd

B, D, N = 8, 1024, 2048
P = 128
CE = D // P   # 8 chunks on the feature axis
CN = N // P   # 16 chunks on the sequence axis
K_SCALE = 1.0 / float(np.sqrt(D))

F32 = mybir.dt.float32
BF16 = mybir.dt.bfloat16


def build_nc():
    nc = bacc.Bacc("TRN2", target_bir_lowering=False, debug=False)

    x_ext = nc.dram_tensor("x", [D, N], F32, kind="ExternalInput")
    wq_ext = nc.dram_tensor("W_q", [D, D], F32, kind="ExternalInput")
    wk_ext = nc.dram_tensor("W_k", [D, D], F32, kind="ExternalInput")
    wv_ext = nc.dram_tensor("W_v", [D, D], F32, kind="ExternalInput")
    out_ext = nc.dram_tensor("out", [D, N], F32, kind="ExternalOutput")

    x_re = x_ext.ap().rearrange("(c p) n -> c p n", p=P)
    wq_re = wq_ext.ap().rearrange("(c p) e -> c p e", p=P)
    wk_re = wk_ext.ap().rearrange("(c p) e -> c p e", p=P)
    wv_re = wv_ext.ap().rearrange("(c p) e -> c p e", p=P)
    out_re = out_ext.ap().rearrange("(c p) m -> c p m", p=P)

    with tile.TileContext(nc) as tc:
        with (
            tc.tile_pool(name="const", bufs=1) as const,
            tc.tile_pool(name="stage", bufs=3) as stage,
            tc.tile_pool(name="big", bufs=21) as big,
            tc.tile_pool(name="small", bufs=4) as small,
            tc.tile_pool(name="dram", bufs=1, space="DRAM") as dram,
            tc.tile_pool(name="psum", bufs=4, space="PSUM") as psum,
        ):
            recip_z = const.tile([P, CN], F32, tag="rz")

            # All big tensors share one 8KB/partition slot tag so SBUF slots
            # recycle across phases (peak ~20 live of 21 slots).
            wq_t = [big.tile([P, 4, 1024], BF16, tag="big", name=f"wq{i}") for i in range(2)]
            wk_t = [big.tile([P, 4, 1024], BF16, tag="big", name=f"wk{i}") for i in range(2)]
            wvt_t = [big.tile([P, 4, 1024], BF16, tag="big", name=f"wvt{i}") for i in range(2)]
            gt_t = [big.tile([P, 4, 1024], BF16, tag="big", name=f"gt{i}") for i in range(2)]
            x_t = [big.tile([P, 2, N], BF16, tag="big", name=f"x{i}") for i in range(4)]
            u_t = [big.tile([P, 2, N], BF16, tag="big", name=f"u{i}") for i in range(4)]
            vt_t = [big.tile([P, 4, 1024], BF16, tag="big", name=f"vt{i}") for i in range(4)]
            attn_t = [big.tile([P, 2, N], BF16, tag="big", name=f"at{i}") for i in range(8)]

            _ps_n = [0]

            def ps_tile():
                _ps_n[0] += 1
                return psum.tile(
                    [P, 1024], F32, tag="ps", bufs=4, name=f"ps{_ps_n[0]}"
                )

            # ---- load + cast W_q, W_k (interleaved so GT starts early) -----
            for c in range(CE):
                for w_re_, w_t in ((wq_re, wq_t), (wk_re, wk_t)):
                    st = stage.tile([P, D], F32, tag="stage")
                    nc.sync.dma_start(st[:], w_re_[c])
                    nc.vector.tensor_copy(w_t[c // 4][:, c % 4, :], st[:])

            # ---- load + cast x [e, n] --------------------------------------
            for c in range(CE):
                st = stage.tile([P, N], F32, tag="stage")
                nc.sync.dma_start(st[:], x_re[c])
                nc.vector.tensor_copy(x_t[c // 2][:, c % 2, :], st[:])

            # ---- W_v: load, cast, XBAR-transpose via DRAM scratch ----------
            wv_scratch = dram.tile([D, D], BF16)
            for cd in range(CE):
                st = stage.tile([P, D], F32, tag="stage")
                nc.sync.dma_start(st[:], wv_re[cd])
                wvc = small.tile([P, D], BF16, tag="wvc", bufs=2)
                nc.vector.tensor_copy(wvc[:], st[:])
                nc.sync.dma_start(wv_scratch[cd * P:(cd + 1) * P, :], wvc[:])
            for ce in range(CE):
                nc.sync.dma_start(
                    out=wvt_t[ce // 4][:, ce % 4, :],
                    in_=wv_scratch[:, ce * P:(ce + 1) * P],
                    transpose=True,
                )

            # ---- GT = W_k^T W_q  [f, e] ------------------------------------
            for cf in range(CE):
                ps = ps_tile()
                for dc in range(CE):
                    for et in range(2):
                        nc.tensor.matmul(
                            ps[:, et * 512:(et + 1) * 512],
                            wk_t[dc // 4][:, dc % 4, cf * P:(cf + 1) * P],
                            wq_t[dc // 4][:, dc % 4, et * 512:(et + 1) * 512],
                            start=(dc == 0),
                            stop=(dc == CE - 1),
                        )
                nc.vector.tensor_copy(gt_t[cf // 4][:, cf % 4, :], ps[:])

            # ---- U = GT^T x = (W_q^T W_k) x  [e, m] ------------------------
            for ce in range(CE):
                for mh in range(2):
                    ps = ps_tile()
                    for cf in range(CE):
                        for mq in range(2):
                            m0 = (mh * 2 + mq) * 512
                            nc.tensor.matmul(
                                ps[:, mq * 512:(mq + 1) * 512],
                                gt_t[cf // 4][:, cf % 4, ce * P:(ce + 1) * P],
                                x_t[cf // 2][:, cf % 2, m0:m0 + 512],
                                start=(cf == 0),
                                stop=(cf == CE - 1),
                            )
                    nc.vector.tensor_copy(
                        u_t[ce // 2][:, ce % 2, mh * 1024:(mh + 1) * 1024], ps[:]
                    )

            # ---- VT = x^T WvT  [n, d] --------------------------------------
            for cn in range(CN):
                ps = ps_tile()
                for ce in range(CE):
                    for dt in range(2):
                        nc.tensor.matmul(
                            ps[:, dt * 512:(dt + 1) * 512],
                            x_t[ce // 2][:, ce % 2, cn * P:(cn + 1) * P],
                            wvt_t[ce // 4][:, ce % 4, dt * 512:(dt + 1) * 512],
                            start=(ce == 0),
                            stop=(ce == CE - 1),
                        )
                nc.vector.tensor_copy(vt_t[cn // 4][:, cn % 4, :], ps[:])

            # ---- scores + softmax per 128-row chunk ------------------------
            for cn in range(CN):
                halves = [ps_tile(), ps_tile()]
                for ce in range(CE):
                    for h in range(2):
                        for ms in range(2):
                            m0 = h * 1024 + ms * 512
                            nc.tensor.matmul(
                                halves[h][:, ms * 512:(ms + 1) * 512],
                                x_t[ce // 2][:, ce % 2, cn * P:(cn + 1) * P],
                                u_t[ce // 2][:, ce % 2, m0:m0 + 512],
                                start=(ce == 0),
                                stop=(ce == CE - 1),
                            )
                mx0 = small.tile([P, 1], F32, tag="sm", bufs=4)
                mx1 = small.tile([P, 1], F32, tag="sm", bufs=4)
                nc.vector.reduce_max(mx0[:], halves[0][:], axis=mybir.AxisListType.X)
                nc.vector.reduce_max(mx1[:], halves[1][:], axis=mybir.AxisListType.X)
                bias = small.tile([P, 1], F32, tag="sm", bufs=4)
                nc.vector.tensor_max(bias[:], mx0[:], mx1[:])
                nc.vector.tensor_scalar_mul(bias[:], bias[:], -K_SCALE)
                z0 = small.tile([P, 1], F32, tag="sm", bufs=4)
                z1 = small.tile([P, 1], F32, tag="sm", bufs=4)
                for h, zh in ((0, z0), (1, z1)):
                    nc.scalar.activation(
                        attn_t[cn // 2][:, cn % 2, h * 1024:(h + 1) * 1024],
                        halves[h][:],
                        mybir.ActivationFunctionType.Exp,
                        bias=bias[:],
                        scale=K_SCALE,
                        accum_out=zh[:],
                    )
                nc.vector.tensor_add(z0[:], z0[:], z1[:])
                nc.vector.reciprocal(recip_z[:, cn:cn + 1], z0[:])

            # ---- fold 1/Z into VT rows -------------------------------------
            for cn in range(CN):
                nc.vector.tensor_scalar_mul(
                    vt_t[cn // 4][:, cn % 4, :],
                    vt_t[cn // 4][:, cn % 4, :],
                    recip_z[:, cn:cn + 1],
                )

            # ---- out = VTs^T attn  [d, m] ----------------------------------
            for dt in range(CE):
                ot = stage.tile([P, N], F32, tag="stage")
                for mh in range(2):
                    ps = ps_tile()
                    for cn in range(CN):
                        for mq in range(2):
                            m0 = (mh * 2 + mq) * 512
                            nc.tensor.matmul(
                                ps[:, mq * 512:(mq + 1) * 512],
                                vt_t[cn // 4][:, cn % 4, dt * P:(dt + 1) * P],
                                attn_t[cn // 2][:, cn % 2, m0:m0 + 512],
                                start=(cn == 0),
                                stop=(cn == CN - 1),
                            )
                    nc.vector.tensor_copy(ot[:, mh * 1024:(mh + 1) * 1024], ps[:])
                    nc.sync.dma_start(
                        out_re[dt][:, mh * 1024:(mh + 1) * 1024],
                        ot[:, mh * 1024:(mh + 1) * 1024],
                    )

    nc.compile()
    return nc


_NC = None


def _get_nc():
    global _NC
    if _NC is None:
        _NC = build_nc()
    return _NC


def kernel(x, W_q, W_k, W_v):
    x = np.ascontiguousarray(np.asarray(x, dtype=np.float32))
    W_q = np.ascontiguousarray(np.asarray(W_q, dtype=np.float32))
    W_k = np.ascontiguousarray(np.asarray(W_k, dtype=np.float32))
    W_v = np.ascontiguousarray(np.asarray(W_v, dtype=np.float32))
    assert x.shape == (B, D, N), x.shape

    nc = _get_nc()
    in_maps = [
        {"x": x[i], "W_q": W_q, "W_k": W_k, "W_v": W_v} for i in range(B)
    ]
    res = run_bass_kernel_spmd(nc, in_maps, core_ids=list(range(B)))
    return np.stack([res.results[i]["out"] for i in range(B)], axis=0)


if __name__ == "__main__":
    rng = np.random.default_rng(0)
    scale = 1.0 / np.sqrt(D)
    x = rng.standard_normal((B, D, N), dtype=np.float32)
    wq = rng.standard_normal((D, D), dtype=np.float32) * scale
    wk = rng.standard_normal((D, D), dtype=np.float32) * scale
    wv = rng.standard_normal((D, D), dtype=np.float32) * scale
    out = kernel(x, wq, wk, wv)
    print("out", out.shape, out.dtype, np.abs(out).max())


# revision 6
# speedup vs baseline: 1.0247x; 1.0057x over previous
"""Single-head attention (B=8, D=1024, N=2048, fp32 I/O) on 8 TRN2 NeuronCores.

Sharding: data-parallel over batch — core i computes batch element i with the
full weights replicated. No collectives needed.

Per-core math (x: [D, N] features-first, W*: [D, D]):
    scores = x^T (W_q^T W_k) x / sqrt(D)        (Gram-matrix trick: no weight
    attn   = softmax(scores, axis=-1)            transposes needed on the Q/K
    out    = (W_v x) attn                        path)
  GT = W_k^T W_q              -> matmul(lhsT=W_k, rhs=W_q)     [f, e]
  U  = GT^T x = (W_q^T W_k) x -> matmul(lhsT=GT, rhs=x)        [e, m]
  S  = x^T U                  -> matmul(lhsT=x,  rhs=U)        [n, m]
  VT = x^T W_v^T              -> matmul(lhsT=x,  rhs=W_v^T)    [n, d]
  out= VT^T attn              -> matmul(lhsT=VT, rhs=attn)     [d, m]
W_v^T comes from an XBAR DMA transpose of the bf16 W_v via a DRAM scratch
(keeps the transpose off the TensorEngine). The softmax normalizer 1/Z[n] is
folded into VT's rows (n is the contraction index of the output matmul), so
attn is stored as unnormalized exp() in bf16. Compute dtype bf16 (f32 PSUM
accumulation); measured rel_l2 vs the f32 reference ~5e-3.

All PSUM tiles share one [128, 1024] (2-bank) tag, 4 bufs = all 8 banks; each
tile carries two interleaved 512-wide accumulation groups (separate banks) so
DMA-gated phases keep several matmuls issuable per arriving input chunk.
"""

import numpy as np

import concourse.bacc as bacc
import concourse.mybir as mybir
import concourse.tile as tile
from concourse.bass_utils import run_bass_kernel_spmd

B, D, N = 8, 1024, 2048
P = 128
CE = D // P   # 8 chunks on the feature axis
CN = N // P   # 16 chunks on the sequence axis
K_SCALE = 1.0 / float(np.sqrt(D))

F32 = mybir.dt.float32
BF16 = mybir.dt.bfloat16


def build_nc():
    nc = bacc.Bacc("TRN2", target_bir_lowering=False, debug=False)

    x_ext = nc.dram_tensor("x", [D, N], F32, kind="ExternalInput")
    wq_ext = nc.dram_tensor("W_q", [D, D], F32, kind="ExternalInput")
    wk_ext = nc.dram_tensor("W_k", [D, D], F32, kind="ExternalInput")
    wv_ext = nc.dram_tensor("W_v", [D, D], F32, kind="ExternalInput")
    out_ext = nc.dram_tensor("out", [D, N], F32, kind="ExternalOutput")

    x_re = x_ext.ap().rearrange("(c p) n -> c p n", p=P)
    wq_re = wq_ext.ap().rearrange("(c p) e -> c p e", p=P)
    wk_re = wk_ext.ap().rearrange("(c p) e -> c p e", p=P)
    wv_re = wv_ext.ap().rearrange("(c p) e -> c p e", p=P)
    out_re = out_ext.ap().rearrange("(c p) m -> c p m", p=P)

    with tile.TileContext(nc) as tc:
        with (
            tc.tile_pool(name="const", bufs=1) as const,
            tc.tile_pool(name="stage", bufs=3) as stage,
            tc.tile_pool(name="big", bufs=21) as big,
            tc.tile_pool(name="small", bufs=4) as small,
            tc.tile_pool(name="dram", bufs=1, space="DRAM") as dram,
            tc.tile_pool(name="psum", bufs=4, space="PSUM") as psum,
        ):
            recip_z = const.tile([P, CN], F32, tag="rz")

            # All big tensors share one 8KB/partition slot tag so SBUF slots
            # recycle across phases (peak ~20 live of 21 slots).
            wq_t = [big.tile([P, 4, 1024], BF16, tag="big", name=f"wq{i}") for i in range(2)]
            wk_t = [big.tile([P, 4, 1024], BF16, tag="big", name=f"wk{i}") for i in range(2)]
            wvt_t = [big.tile([P, 4, 1024], BF16, tag="big", name=f"wvt{i}") for i in range(2)]
            gt_t = [big.tile([P, 4, 1024], BF16, tag="big", name=f"gt{i}") for i in range(2)]
            x_t = [big.tile([P, 2, N], BF16, tag="big", name=f"x{i}") for i in range(4)]
            u_t = [big.tile([P, 2, N], BF16, tag="big", name=f"u{i}") for i in range(4)]
            vt_t = [big.tile([P, 4, 1024], BF16, tag="big", name=f"vt{i}") for i in range(4)]
            attn_t = [big.tile([P, 2, N], BF16, tag="big", name=f"at{i}") for i in range(8)]

            _ps_n = [0]

            def ps_tile():
                _ps_n[0] += 1
                return psum.tile(
                    [P, 1024], F32, tag="ps", bufs=4, name=f"ps{_ps_n[0]}"
                )

            # ---- load + cast W_q, W_k (interleaved so GT starts early) -----
            for c in range(CE):
                for w_re_, w_t in ((wq_re, wq_t), (wk_re, wk_t)):
                    st = stage.tile([P, D], F32, tag="stage")
                    nc.sync.dma_start(st[:], w_re_[c])
                    nc.vector.tensor_copy(w_t[c // 4][:, c % 4, :], st[:])

            # ---- load + cast x [e, n] --------------------------------------
            for c in range(CE):
                st = stage.tile([P, N], F32, tag="stage")
                nc.sync.dma_start(st[:], x_re[c])
                nc.vector.tensor_copy(x_t[c // 2][:, c % 2, :], st[:])

            # ---- W_v: load, cast, XBAR-transpose via DRAM scratch ----------
            wv_scratch = dram.tile([D, D], BF16)
            for cd in range(CE):
                st = stage.tile([P, D], F32, tag="stage")
                nc.sync.dma_start(st[:], wv_re[cd])
                wvc = small.tile([P, D], BF16, tag="wvc", bufs=2)
                nc.vector.tensor_copy(wvc[:], st[:])
                nc.sync.dma_start(wv_scratch[cd * P:(cd + 1) * P, :], wvc[:])
            for ce in range(CE):
                nc.sync.dma_start(
                    out=wvt_t[ce // 4][:, ce % 4, :],
                    in_=wv_scratch[:, ce * P:(ce + 1) * P],
                    transpose=True,
                )

            # ---- GT = W_k^T W_q  [f, e] ------------------------------------
            # Waves of 4 psum tiles with the contraction (dc) loop outermost:
            # each arriving W chunk pair feeds 8 issuable matmuls, so the
            # in-order PE stream isn't head-of-line blocked on late chunks.
            for wave in range(2):
                cfs = range(wave * 4, wave * 4 + 4)
                tiles = {cf: ps_tile() for cf in cfs}
                for dc in range(CE):
                    for cf in cfs:
                        for et in range(2):
                            nc.tensor.matmul(
                                tiles[cf][:, et * 512:(et + 1) * 512],
                                wk_t[dc // 4][:, dc % 4, cf * P:(cf + 1) * P],
                                wq_t[dc // 4][:, dc % 4, et * 512:(et + 1) * 512],
                                start=(dc == 0),
                                stop=(dc == CE - 1),
                            )
                for cf in cfs:
                    nc.vector.tensor_copy(gt_t[cf // 4][:, cf % 4, :], tiles[cf][:])

            # ---- U = GT^T x = (W_q^T W_k) x  [e, m] ------------------------
            for wave in range(4):
                ces = (2 * wave, 2 * wave + 1)
                tiles = {(ce, mh): ps_tile() for ce in ces for mh in range(2)}
                for cf in range(CE):
                    for ce in ces:
                        for mh in range(2):
                            for mq in range(2):
                                m0 = (mh * 2 + mq) * 512
                                nc.tensor.matmul(
                                    tiles[ce, mh][:, mq * 512:(mq + 1) * 512],
                                    gt_t[cf // 4][:, cf % 4, ce * P:(ce + 1) * P],
                                    x_t[cf // 2][:, cf % 2, m0:m0 + 512],
                                    start=(cf == 0),
                                    stop=(cf == CE - 1),
                                )
                for (ce, mh), ps in tiles.items():
                    nc.vector.tensor_copy(
                        u_t[ce // 2][:, ce % 2, mh * 1024:(mh + 1) * 1024], ps[:]
                    )

            # ---- scores + softmax per 128-row chunk ------------------------
            for cn in range(CN):
                halves = [ps_tile(), ps_tile()]
                for ce in range(CE):
                    for h in range(2):
                        for ms in range(2):
                            m0 = h * 1024 + ms * 512
                            nc.tensor.matmul(
                                halves[h][:, ms * 512:(ms + 1) * 512],
                                x_t[ce // 2][:, ce % 2, cn * P:(cn + 1) * P],
                                u_t[ce // 2][:, ce % 2, m0:m0 + 512],
                                start=(ce == 0),
                                stop=(ce == CE - 1),
                            )
                mx0 = small.tile([P, 1], F32, tag="sm", bufs=4)
                mx1 = small.tile([P, 1], F32, tag="sm", bufs=4)
                nc.vector.reduce_max(mx0[:], halves[0][:], axis=mybir.AxisListType.X)
                nc.vector.reduce_max(mx1[:], halves[1][:], axis=mybir.AxisListType.X)
                bias = small.tile([P, 1], F32, tag="sm", bufs=4)
                nc.vector.tensor_max(bias[:], mx0[:], mx1[:])
                nc.vector.tensor_scalar_mul(bias[:], bias[:], -K_SCALE)
                z0 = small.tile([P, 1], F32, tag="sm", bufs=4)
                z1 = small.tile([P, 1], F32, tag="sm", bufs=4)
                for h, zh in ((0, z0), (1, z1)):
                    nc.scalar.activation(
                        attn_t[cn // 2][:, cn % 2, h * 1024:(h + 1) * 1024],
                        halves[h][:],
                        mybir.ActivationFunctionType.Exp,
                        bias=bias[:],
                        scale=K_SCALE,
                        accum_out=zh[:],
                    )
                nc.vector.tensor_add(z0[:], z0[:], z1[:])
                nc.vector.reciprocal(recip_z[:, cn:cn + 1], z0[:])

            # ---- VT = x^T WvT  [n, d] --------------------------------------
            # After scores in the PE stream: WvT's DMA transpose arrives late
            # in the load queue, and nothing before AV needs VT.
            for cn in range(CN):
                ps = ps_tile()
                for ce in range(CE):
                    for dt in range(2):
                        nc.tensor.matmul(
                            ps[:, dt * 512:(dt + 1) * 512],
                            x_t[ce // 2][:, ce % 2, cn * P:(cn + 1) * P],
                            wvt_t[ce // 4][:, ce % 4, dt * 512:(dt + 1) * 512],
                            start=(ce == 0),
                            stop=(ce == CE - 1),
                        )
                nc.vector.tensor_copy(vt_t[cn // 4][:, cn % 4, :], ps[:])

            # ---- fold 1/Z into VT rows -------------------------------------
            for cn in range(CN):
                nc.vector.tensor_scalar_mul(
                    vt_t[cn // 4][:, cn % 4, :],
                    vt_t[cn // 4][:, cn % 4, :],
                    recip_z[:, cn:cn + 1],
                )

            # ---- out = VTs^T attn  [d, m] ----------------------------------
            for dt in range(CE):
                ot = stage.tile([P, N], F32, tag="stage")
                for mh in range(2):
                    ps = ps_tile()
                    for cn in range(CN):
                        for mq in range(2):
                            m0 = (mh * 2 + mq) * 512
                            nc.tensor.matmul(
                                ps[:, mq * 512:(mq + 1) * 512],
                                vt_t[cn // 4][:, cn % 4, dt * P:(dt + 1) * P],
                                attn_t[cn // 2][:, cn % 2, m0:m0 + 512],
                                start=(cn == 0),
                                stop=(cn == CN - 1),
                            )
                    nc.vector.tensor_copy(ot[:, mh * 1024:(mh + 1) * 1024], ps[:])
                    nc.sync.dma_start(
                        out_re[dt][:, mh * 1024:(mh + 1) * 1024],
                        ot[:, mh * 1024:(mh + 1) * 1024],
                    )

    nc.compile()
    return nc


_NC = None


def _get_nc():
    global _NC
    if _NC is None:
        _NC = build_nc()
    return _NC


def kernel(x, W_q, W_k, W_v):
    x = np.ascontiguousarray(np.asarray(x, dtype=np.float32))
    W_q = np.ascontiguousarray(np.asarray(W_q, dtype=np.float32))
    W_k = np.ascontiguousarray(np.asarray(W_k, dtype=np.float32))
    W_v = np.ascontiguousarray(np.asarray(W_v, dtype=np.float32))
    assert x.shape == (B, D, N), x.shape

    nc = _get_nc()
    in_maps = [
        {"x": x[i], "W_q": W_q, "W_k": W_k, "W_v": W_v} for i in range(B)
    ]
    res = run_bass_kernel_spmd(nc, in_maps, core_ids=list(range(B)))
    return np.stack([res.results[i]["out"] for i in range(B)], axis=0)


if __name__ == "__main__":
    rng = np.random.default_rng(0)
    scale = 1.0 / np.sqrt(D)
    x = rng.standard_normal((B, D, N), dtype=np.float32)
    wq = rng.standard_normal((D, D), dtype=np.float32) * scale
    wk = rng.standard_normal((D, D), dtype=np.float32) * scale
    wv = rng.standard_normal((D, D), dtype=np.float32) * scale
    out = kernel(x, wq, wk, wv)
    print("out", out.shape, out.dtype, np.abs(out).max())


# revision 7
# speedup vs baseline: 1.1208x; 1.0938x over previous
"""Single-head attention (B=8, D=1024, N=2048, fp32 I/O) on 8 TRN2 NeuronCores.

Sharding: data-parallel over batch — core i computes batch element i with the
full weights replicated. No collectives needed.

Per-core math (x: [D, N] features-first, W*: [D, D]):
    scores = x^T (W_q^T W_k) x / sqrt(D)        (Gram-matrix trick: no weight
    attn   = softmax(scores, axis=-1)            transposes needed on the Q/K
    out    = (W_v x) attn                        path)
  GT = W_k^T W_q              -> matmul(lhsT=W_k, rhs=W_q)     [f, e]
  U  = GT^T x = (W_q^T W_k) x -> matmul(lhsT=GT, rhs=x)        [e, m]
  S  = x^T U                  -> matmul(lhsT=x,  rhs=U)        [n, m]
  VT = x^T W_v^T              -> matmul(lhsT=x,  rhs=W_v^T)    [n, d]
  out= VT^T attn              -> matmul(lhsT=VT, rhs=attn)     [d, m]
W_v^T comes from an XBAR DMA transpose of the bf16 W_v via a DRAM scratch
(keeps the transpose off the TensorEngine). The softmax normalizer 1/Z[n] is
folded into VT's rows (n is the contraction index of the output matmul), so
attn is stored as unnormalized exp() in bf16. Compute dtype bf16 (f32 PSUM
accumulation); measured rel_l2 vs the f32 reference ~5e-3.

All PSUM tiles share one [128, 1024] (2-bank) tag, 4 bufs = all 8 banks; each
tile carries two interleaved 512-wide accumulation groups (separate banks) so
DMA-gated phases keep several matmuls issuable per arriving input chunk.
"""

import numpy as np

import concourse.bacc as bacc
import concourse.mybir as mybir
import concourse.tile as tile
from concourse.bass_utils import run_bass_kernel_spmd

B, D, N = 8, 1024, 2048
P = 128
CE = D // P   # 8 chunks on the feature axis
CN = N // P   # 16 chunks on the sequence axis
K_SCALE = 1.0 / float(np.sqrt(D))

F32 = mybir.dt.float32
BF16 = mybir.dt.bfloat16


def build_nc():
    nc = bacc.Bacc("TRN2", target_bir_lowering=False, debug=False)

    x_ext = nc.dram_tensor("x", [D, N], F32, kind="ExternalInput")
    wq_ext = nc.dram_tensor("W_q", [D, D], F32, kind="ExternalInput")
    wk_ext = nc.dram_tensor("W_k", [D, D], F32, kind="ExternalInput")
    wv_ext = nc.dram_tensor("W_v", [D, D], F32, kind="ExternalInput")
    out_ext = nc.dram_tensor("out", [D, N], F32, kind="ExternalOutput")

    x_re = x_ext.ap().rearrange("(c p) n -> c p n", p=P)
    wq_re = wq_ext.ap().rearrange("(c p) e -> c p e", p=P)
    wk_re = wk_ext.ap().rearrange("(c p) e -> c p e", p=P)
    wv_re = wv_ext.ap().rearrange("(c p) e -> c p e", p=P)
    out_re = out_ext.ap().rearrange("(c p) m -> c p m", p=P)

    with tile.TileContext(nc) as tc:
        with (
            tc.tile_pool(name="const", bufs=1) as const,
            tc.tile_pool(name="stage", bufs=3) as stage,
            tc.tile_pool(name="big", bufs=21) as big,
            tc.tile_pool(name="small", bufs=4) as small,
            tc.tile_pool(name="dram", bufs=1, space="DRAM") as dram,
            tc.tile_pool(name="psum", bufs=4, space="PSUM") as psum,
        ):
            recip_z = const.tile([P, CN], F32, tag="rz")

            # All big tensors share one 8KB/partition slot tag so SBUF slots
            # recycle across phases (peak ~20 live of 21 slots).
            wq_t = [big.tile([P, 4, 1024], BF16, tag="big", name=f"wq{i}") for i in range(2)]
            wk_t = [big.tile([P, 4, 1024], BF16, tag="big", name=f"wk{i}") for i in range(2)]
            wvt_t = [big.tile([P, 4, 1024], BF16, tag="big", name=f"wvt{i}") for i in range(2)]
            gt_t = [big.tile([P, 4, 1024], BF16, tag="big", name=f"gt{i}") for i in range(2)]
            x_t = [big.tile([P, 2, N], BF16, tag="big", name=f"x{i}") for i in range(4)]
            u_t = [big.tile([P, 2, N], BF16, tag="big", name=f"u{i}") for i in range(4)]
            vt_t = [big.tile([P, 4, 1024], BF16, tag="big", name=f"vt{i}") for i in range(4)]
            attn_t = [big.tile([P, 2, N], BF16, tag="big", name=f"at{i}") for i in range(8)]

            _ps_n = [0]

            def ps_tile():
                _ps_n[0] += 1
                return psum.tile(
                    [P, 1024], F32, tag="ps", bufs=4, name=f"ps{_ps_n[0]}"
                )

            # ---- load + cast W_q, W_k (interleaved so GT starts early) -----
            for c in range(CE):
                for w_re_, w_t in ((wq_re, wq_t), (wk_re, wk_t)):
                    st = stage.tile([P, D], F32, tag="stage")
                    nc.sync.dma_start(st[:], w_re_[c])
                    nc.vector.tensor_copy(w_t[c // 4][:, c % 4, :], st[:])

            # ---- load + cast x [e, n] --------------------------------------
            # Casts on the (otherwise idle) scalar engine: the in-order
            # vector engine must not head-of-line block the GT/U psum evicts
            # behind these DMA-gated casts.
            for c in range(CE):
                st = stage.tile([P, N], F32, tag="stage")
                nc.sync.dma_start(st[:], x_re[c])
                nc.scalar.copy(x_t[c // 2][:, c % 2, :], st[:])

            # ---- W_v: load, cast, XBAR-transpose via DRAM scratch ----------
            wv_scratch = dram.tile([D, D], BF16)
            for cd in range(CE):
                st = stage.tile([P, D], F32, tag="stage")
                nc.sync.dma_start(st[:], wv_re[cd])
                wvc = small.tile([P, D], BF16, tag="wvc", bufs=2)
                nc.scalar.copy(wvc[:], st[:])
                nc.sync.dma_start(wv_scratch[cd * P:(cd + 1) * P, :], wvc[:])
            for ce in range(CE):
                nc.sync.dma_start(
                    out=wvt_t[ce // 4][:, ce % 4, :],
                    in_=wv_scratch[:, ce * P:(ce + 1) * P],
                    transpose=True,
                )

            # ---- GT = W_k^T W_q  [f, e] ------------------------------------
            # Waves of 4 psum tiles with the contraction (dc) loop outermost:
            # each arriving W chunk pair feeds 8 issuable matmuls, so the
            # in-order PE stream isn't head-of-line blocked on late chunks.
            for wave in range(2):
                cfs = range(wave * 4, wave * 4 + 4)
                tiles = {cf: ps_tile() for cf in cfs}
                for dc in range(CE):
                    for cf in cfs:
                        for et in range(2):
                            nc.tensor.matmul(
                                tiles[cf][:, et * 512:(et + 1) * 512],
                                wk_t[dc // 4][:, dc % 4, cf * P:(cf + 1) * P],
                                wq_t[dc // 4][:, dc % 4, et * 512:(et + 1) * 512],
                                start=(dc == 0),
                                stop=(dc == CE - 1),
                            )
                for cf in cfs:
                    nc.vector.tensor_copy(gt_t[cf // 4][:, cf % 4, :], tiles[cf][:])

            # ---- U = GT^T x = (W_q^T W_k) x  [e, m] ------------------------
            for wave in range(4):
                ces = (2 * wave, 2 * wave + 1)
                tiles = {(ce, mh): ps_tile() for ce in ces for mh in range(2)}
                for cf in range(CE):
                    for ce in ces:
                        for mh in range(2):
                            for mq in range(2):
                                m0 = (mh * 2 + mq) * 512
                                nc.tensor.matmul(
                                    tiles[ce, mh][:, mq * 512:(mq + 1) * 512],
                                    gt_t[cf // 4][:, cf % 4, ce * P:(ce + 1) * P],
                                    x_t[cf // 2][:, cf % 2, m0:m0 + 512],
                                    start=(cf == 0),
                                    stop=(cf == CE - 1),
                                )
                for (ce, mh), ps in tiles.items():
                    nc.vector.tensor_copy(
                        u_t[ce // 2][:, ce % 2, mh * 1024:(mh + 1) * 1024], ps[:]
                    )

            # ---- scores + softmax per 128-row chunk ------------------------
            for cn in range(CN):
                halves = [ps_tile(), ps_tile()]
                for ce in range(CE):
                    for h in range(2):
                        for ms in range(2):
                            m0 = h * 1024 + ms * 512
                            nc.tensor.matmul(
                                halves[h][:, ms * 512:(ms + 1) * 512],
                                x_t[ce // 2][:, ce % 2, cn * P:(cn + 1) * P],
                                u_t[ce // 2][:, ce % 2, m0:m0 + 512],
                                start=(ce == 0),
                                stop=(ce == CE - 1),
                            )
                mx0 = small.tile([P, 1], F32, tag="sm", bufs=4)
                mx1 = small.tile([P, 1], F32, tag="sm", bufs=4)
                nc.vector.reduce_max(mx0[:], halves[0][:], axis=mybir.AxisListType.X)
                nc.vector.reduce_max(mx1[:], halves[1][:], axis=mybir.AxisListType.X)
                bias = small.tile([P, 1], F32, tag="sm", bufs=4)
                nc.vector.tensor_max(bias[:], mx0[:], mx1[:])
                nc.vector.tensor_scalar_mul(bias[:], bias[:], -K_SCALE)
                z0 = small.tile([P, 1], F32, tag="sm", bufs=4)
                z1 = small.tile([P, 1], F32, tag="sm", bufs=4)
                for h, zh in ((0, z0), (1, z1)):
                    nc.scalar.activation(
                        attn_t[cn // 2][:, cn % 2, h * 1024:(h + 1) * 1024],
                        halves[h][:],
                        mybir.ActivationFunctionType.Exp,
                        bias=bias[:],
                        scale=K_SCALE,
                        accum_out=zh[:],
                    )
                nc.vector.tensor_add(z0[:], z0[:], z1[:])
                nc.vector.reciprocal(recip_z[:, cn:cn + 1], z0[:])

            # ---- VT = x^T WvT  [n, d] --------------------------------------
            # After scores in the PE stream: WvT's DMA transpose arrives late
            # in the load queue, and nothing before AV needs VT.
            for cn in range(CN):
                ps = ps_tile()
                for ce in range(CE):
                    for dt in range(2):
                        nc.tensor.matmul(
                            ps[:, dt * 512:(dt + 1) * 512],
                            x_t[ce // 2][:, ce % 2, cn * P:(cn + 1) * P],
                            wvt_t[ce // 4][:, ce % 4, dt * 512:(dt + 1) * 512],
                            start=(ce == 0),
                            stop=(ce == CE - 1),
                        )
                nc.vector.tensor_copy(vt_t[cn // 4][:, cn % 4, :], ps[:])

            # ---- fold 1/Z into VT rows -------------------------------------
            for cn in range(CN):
                nc.vector.tensor_scalar_mul(
                    vt_t[cn // 4][:, cn % 4, :],
                    vt_t[cn // 4][:, cn % 4, :],
                    recip_z[:, cn:cn + 1],
                )

            # ---- out = VTs^T attn  [d, m] ----------------------------------
            for dt in range(CE):
                ot = stage.tile([P, N], F32, tag="stage")
                for mh in range(2):
                    ps = ps_tile()
                    for cn in range(CN):
                        for mq in range(2):
                            m0 = (mh * 2 + mq) * 512
                            nc.tensor.matmul(
                                ps[:, mq * 512:(mq + 1) * 512],
                                vt_t[cn // 4][:, cn % 4, dt * P:(dt + 1) * P],
                                attn_t[cn // 2][:, cn % 2, m0:m0 + 512],
                                start=(cn == 0),
                                stop=(cn == CN - 1),
                            )
                    nc.vector.tensor_copy(ot[:, mh * 1024:(mh + 1) * 1024], ps[:])
                    nc.sync.dma_start(
                        out_re[dt][:, mh * 1024:(mh + 1) * 1024],
                        ot[:, mh * 1024:(mh + 1) * 1024],
                    )

    nc.compile()
    return nc


_NC = None


def _get_nc():
    global _NC
    if _NC is None:
        _NC = build_nc()
    return _NC


def kernel(x, W_q, W_k, W_v):
    x = np.ascontiguousarray(np.asarray(x, dtype=np.float32))
    W_q = np.ascontiguousarray(np.asarray(W_q, dtype=np.float32))
    W_k = np.ascontiguousarray(np.asarray(W_k, dtype=np.float32))
    W_v = np.ascontiguousarray(np.asarray(W_v, dtype=np.float32))
    assert x.shape == (B, D, N), x.shape

    nc = _get_nc()
    in_maps = [
        {"x": x[i], "W_q": W_q, "W_k": W_k, "W_v": W_v} for i in range(B)
    ]
    res = run_bass_kernel_spmd(nc, in_maps, core_ids=list(range(B)))
    return np.stack([res.results[i]["out"] for i in range(B)], axis=0)


if __name__ == "__main__":
    rng = np.random.default_rng(0)
    scale = 1.0 / np.sqrt(D)
    x = rng.standard_normal((B, D, N), dtype=np.float32)
    wq = rng.standard_normal((D, D), dtype=np.float32) * scale
    wk = rng.standard_normal((D, D), dtype=np.float32) * scale
    wv = rng.standard_normal((D, D), dtype=np.float32) * scale
    out = kernel(x, wq, wk, wv)
    print("out", out.shape, out.dtype, np.abs(out).max())
